# revision 4
# baseline (speedup 1.0000x reference)
"""Trainium2 Bass kernel for nn_CausalSelfAttentionSynapse.

Math (per reference):
    qk = g @ W_lift.T                       # (B,T,2E)
    q,k heads of dim D=64; scores = q@k.T causal-masked
    lse[b,h,t] = logsumexp_{j<=t} scores[b,h,t,j]
    out[b,t]  = sum_h lse[b,h,t] * w[h],  w[h] = sum_g W_proj[g,h]

Sharding: 8 cores = 4 batches x 2 head-groups (8 heads each).

Per-core design:
  - Host pre-transposes g[b] / W_lift head-group rows into e-major bf16
    (no on-device transposes, half the DMA bytes). g arrives in 4
    t-quarter tiles, W in 4 head-pair tiles so compute starts early.
  - Lift (bf16 matmuls over e on partitions) lands qkT per head pair in
    score layout: partitions 0-63 head A d, 64-127 head B d; per
    t-quarter tile cols 0:512 q, 512:1024 k.
  - Scores: 64-deep matmuls; paired units alternate base partitions
    0/64 so the PE row-tiles two units concurrently. The causal mask of
    each diagonal block is an identity x strict-upper(-30000) matmul;
    within each PSUM bank all writers form one contiguous accumulation
    group (first start=True, last stop=True) -- has_written only lives
    within an open group.
  - Exp + row-sum fused on ScalarE (accum_out). For deep qtiles the
    first 512/1024 score columns are instead evaluated on VectorE with
    a Schraudolph bitcast exp (x*a+b -> int32 -> reinterpret f32) and a
    batched tensor_reduce, balancing ScalarE vs VectorE.
  - lse = ln(sums) via exponent/mantissa bitcast split + ACT Ln on the
    mantissa; weighted head-sum chain; host adds the 2 head-group
    partials per batch.
"""

import numpy as np
import ml_dtypes

B, T, E, H = 4, 2048, 1024, 16
D = 64
NCORES = 8
NE = E // 128   # 8 e chunks
PAIRS = 4
NQ = T // 128   # 16 query tiles
BIGNEG = -30000.0
SCH_A = float((1 << 23) / np.log(2.0))
SCH_B = float(127 * (1 << 23) - 486408)

_CACHE = {}


def _xsplit(qi):
    if qi < 8:
        return 0
    if qi < 10:
        return 512
    return 1024


def _build():
    import concourse.bass as bass  # noqa: F401
    import concourse.tile as tile
    from concourse import bacc, mybir

    f32 = mybir.dt.float32
    bf16 = mybir.dt.bfloat16
    i32 = mybir.dt.int32
    EXP = mybir.ActivationFunctionType.Exp
    LN = mybir.ActivationFunctionType.Ln
    AX = mybir.AxisListType.X
    MUL = mybir.AluOpType.mult
    ADD = mybir.AluOpType.add
    SHR = mybir.AluOpType.logical_shift_right
    SUB = mybir.AluOpType.subtract
    AND = mybir.AluOpType.bitwise_and
    OR = mybir.AluOpType.bitwise_or
    LN2 = float(np.log(2.0))

    nc = bacc.Bacc("TRN2", target_bir_lowering=False, debug=False,
                   num_devices=NCORES)

    # g_q: quarter tq of g^T: [128, e(8), 512] each, bf16
    g_ds = [nc.dram_tensor(f"g_q{tq}", [128, NE * 512], bf16,
                           kind="ExternalInput").ap() for tq in range(4)]
    # w_p: pair p rows of W^T: [128, e(8), f(256)] each
    w_ds = [nc.dram_tensor(f"w_p{p}", [128, NE * 256], bf16,
                           kind="ExternalInput").ap() for p in range(4)]
    id_d = nc.dram_tensor("ident", [128, 128], bf16,
                          kind="ExternalInput").ap()
    un_d = nc.dram_tensor("uneg", [128, 128], bf16,
                          kind="ExternalInput").ap()
    wb_d = nc.dram_tensor("wb", [128, 8], f32, kind="ExternalInput").ap()
    out_d = nc.dram_tensor("out_part", [128, 16], f32,
                           kind="ExternalOutput").ap()

    with tile.TileContext(nc) as tc:
        with (
            tc.tile_pool(name="consts", bufs=1) as consts,
            tc.tile_pool(name="big", bufs=1) as big,
            tc.tile_pool(name="qkp", bufs=12) as qkp,
            tc.tile_pool(name="exps", bufs=2) as exps,
            tc.tile_pool(name="schp", bufs=2) as schp,
            tc.tile_pool(name="misc", bufs=1) as misc,
            tc.tile_pool(name="fin", bufs=2) as fin,
            tc.tile_pool(name="ps_lift", bufs=2, space="PSUM") as ps_lift,
            tc.tile_pool(name="ps_big", bufs=1, space="PSUM") as ps_big,
            tc.tile_pool(name="ps_small", bufs=1, space="PSUM") as ps_small,
        ):
            # ---- constants & inputs ---------------------------------------
            ident = consts.tile([128, 128], bf16, name="ident", tag="ident")
            nc.sync.dma_start(out=ident[:], in_=id_d[:])
            uneg = consts.tile([128, 128], bf16, name="uneg", tag="uneg")
            nc.sync.dma_start(out=uneg[:], in_=un_d[:])
            wb = consts.tile([128, 8], f32, name="wb", tag="wb")
            nc.sync.dma_start(out=wb[:], in_=wb_d[:])

            # interleave w/g DMAs so pair-0 lift can start earliest
            wT, gq = [], []
            wtiles, gtiles = [], []
            for p in range(4):
                t = big.tile([128, NE * 256], bf16, name=f"wT{p}",
                             tag=f"wT{p}")
                wtiles.append(t)
                wT.append(t.rearrange("p (e f) -> p e f", e=NE))
                t = big.tile([128, NE * 512], bf16, name=f"gq{p}",
                             tag=f"gq{p}")
                gtiles.append(t)
                gq.append(t.rearrange("p (e t) -> p e t", e=NE))
            for i in range(4):
                nc.sync.dma_start(out=wtiles[i][:], in_=w_ds[i][:])
                nc.sync.dma_start(out=gtiles[i][:], in_=g_ds[i][:])

            sums = misc.tile([128, 128], f32, name="sums", tag="sums")
            sumsB = misc.tile([128, 128], f32, name="sumsB", tag="sumsB")
            nc.vector.memset(sumsB[:], 0.0)

            # qk quarter tiles per pair: [128, 1024]: 0:512 q, 512:1024 k
            qk = {}

            def lift_chunk(p, ft, tcq):
                pt = ps_lift.tile([128, 512], f32, name=f"pl{p}{ft}{tcq}",
                                  tag="pslift")
                fo = ft * 128
                for e in range(NE):
                    nc.tensor.matmul(
                        pt[:], lhsT=wT[p][:, e, fo:fo + 128],
                        rhs=gq[tcq][:, e], start=(e == 0), stop=(e == NE - 1))
                nc.vector.tensor_copy(
                    qk[(p, tcq)][:, ft * 512: ft * 512 + 512], pt[:])

            def unit_mms(p, h, qi, pst):
                """Emit score matmuls for one unit; returns nothing."""
                kneed = 128 * (qi + 1)
                kf = (kneed - 128) // 512
                lo, hi = 64 * h, 64 * h + 64
                qq = qi // 4
                lhsT = qk[(p, qq)][lo:hi, (qi % 4) * 128: (qi % 4) * 128 + 128]
                mms = []
                for c in range(kf):
                    mms.append((pst[:, c * 512:(c + 1) * 512], lhsT,
                                qk[(p, c)][lo:hi, 512:1024], True, True))
                # diag-bank group: mask (start) then remainder+diag (stop)
                w = kneed - 512 * kf
                mask = (pst[:, kneed - 128:kneed], ident[:], uneg[:],
                        True, False)
                comb = (pst[:, 512 * kf:kneed], lhsT,
                        qk[(p, kf)][lo:hi, 512:512 + w], False, True)
                return mms, [mask, comb]

            def emit_pair(ua, ub):
                """Interleave two units' chunk MMs (disjoint PE row groups),
                then each unit's diag-bank group contiguously, then the
                exp/sum consumers."""
                tiles = {}
                for (p, h, qi, bigt) in (ua, ub) if ub else (ua,):
                    kneed = 128 * (qi + 1)
                    if bigt:
                        t = ps_big.tile([128, 2048], f32,
                                        name=f"pb{p}{h}{qi}", tag="psbig")
                    else:
                        t = ps_small.tile([128, 1024], f32,
                                          name=f"pc{p}{h}{qi}", tag="pssmall")
                    tiles[(p, h, qi)] = t
                ms = []
                for u in ((ua, ub) if ub else (ua,)):
                    p, h, qi, bigt = u
                    mm, dg = unit_mms(p, h, qi, tiles[(p, h, qi)])
                    ms.append((mm, dg))
                # interleave chunk mms pairwise
                n = max(len(m[0]) for m in ms)
                for i in range(n):
                    for m in ms:
                        if i < len(m[0]):
                            o, l, r, st, sp = m[0][i]
                            nc.tensor.matmul(o, lhsT=l, rhs=r, start=st,
                                             stop=sp)
                for m in ms:
                    for (o, l, r, st, sp) in m[1]:
                        nc.tensor.matmul(o, lhsT=l, rhs=r, start=st, stop=sp)
                # consumers
                for u in ((ua, ub) if ub else (ua,)):
                    p, h, qi, bigt = u
                    pst = tiles[(p, h, qi)]
                    kneed = 128 * (qi + 1)
                    xsp = _xsplit(qi)
                    col = (2 * p + h) * 16 + qi
                    if xsp > 0:
                        _sch_emit(p, h, qi, pst, xsp)
                    eb = exps.tile([128, 2048], bf16, name=f"eb{p}{h}{qi}",
                                   tag="eb")
                    nc.scalar.activation(
                        eb[:, 0:kneed - xsp], pst[:, xsp:kneed], EXP,
                        accum_out=sums[:, col:col + 1])

            # Schraudolph pass1s accumulate into a shared scratch; one
            # batched reduce per 2 units (their sums cols differ by 16).
            sch_state = {"tile": None, "slot": 0, "cols": [], "xsp": 0,
                         "idx": 0}

            def _sch_flush():
                st = sch_state
                if st["tile"] is None or not st["cols"]:
                    return
                xsp = st["xsp"]
                nslot = len(st["cols"])
                v = st["tile"][:].bitcast(f32)
                v3 = v.rearrange("p (u x) -> p u x", u=2)
                c0 = st["cols"][0]
                if nslot == 2:
                    assert st["cols"][1] == c0 + 16
                    out = sumsB[:].rearrange("p (a b) -> p a b", b=16)
                    out2 = out[:, c0 // 16: c0 // 16 + 2, c0 % 16]
                    nc.vector.tensor_reduce(
                        out=out2, in_=v3[:, :, 0:xsp], axis=AX,
                        op=mybir.AluOpType.add)
                else:
                    nc.vector.tensor_reduce(
                        out=sumsB[:, c0:c0 + 1], in_=v3[:, 0, 0:xsp],
                        axis=AX, op=mybir.AluOpType.add)
                st["tile"] = None
                st["slot"] = 0
                st["cols"] = []

            def _sch_emit(p, h, qi, pst, xsp):
                st = sch_state
                if st["tile"] is not None and (st["slot"] == 2
                                               or st["xsp"] != xsp):
                    _sch_flush()
                if st["tile"] is None:
                    st["idx"] += 1
                    st["tile"] = schp.tile([128, 2048], i32,
                                           name=f"sch{st['idx']}", tag="sch")
                    st["xsp"] = xsp
                slot = st["slot"]
                nc.vector.tensor_scalar(
                    out=st["tile"][:, slot * 1024: slot * 1024 + xsp],
                    in0=pst[:, 0:xsp], scalar1=SCH_A, scalar2=SCH_B,
                    op0=MUL, op1=ADD)
                st["cols"].append((2 * p + h) * 16 + qi)
                st["slot"] = slot + 1

            # ---- schedule -------------------------------------------------
            for p in range(4):
                for tcq in range(4):
                    qk[(p, tcq)] = qkp.tile([128, 1024], bf16,
                                            name=f"qk{p}{tcq}", tag="qk")

            lift_q = {p: [(p, ft, tcq) for tcq in range(4) for ft in range(2)]
                      for p in range(4)}

            def pop_lift(p, n=1):
                for _ in range(n):
                    if lift_q[p]:
                        lift_chunk(*lift_q[p].pop(0))

            # prologue: pair 0 lift + its 16 units small-first
            pop_lift(0, 2)
            for i in range(8):
                emit_pair((0, 0, i, False), (0, 1, i, True))
                if i < 6:
                    pop_lift(0, 1)
                if i >= 4:
                    pop_lift(1, 1)

            # stages 0..2: bigs(p) paired with smalls(p+1); stage 3: bigs(3)
            for s in range(3):
                for i in range(8):
                    emit_pair((s, 0, 8 + i, True), (s + 1, 1, i, False))
                    if i < 4:
                        pop_lift(s + 1, 1)
                    elif s < 2:
                        pop_lift(s + 2, 1)
                    emit_pair((s, 1, 8 + i, True), (s + 1, 0, i, False))
                    if s < 2 and i >= 4:
                        pop_lift(s + 2, 1)
            for i in range(8):
                emit_pair((3, 0, 8 + i, True), (3, 1, 8 + i, True))
            _sch_flush()

            # ---- finale ---------------------------------------------------
            tot = fin.tile([128, 128], f32, name="tot", tag="tot")
            nc.vector.tensor_tensor(out=tot[:], in0=sums[:], in1=sumsB[:],
                                    op=ADD)
            u = tot[:].bitcast(i32)
            ei = fin.tile([128, 128], i32, name="ei", tag="ei")
            nc.vector.tensor_scalar(out=ei[:], in0=u, scalar1=23,
                                    scalar2=None, op0=SHR)
            ef = fin.tile([128, 128], f32, name="ef", tag="ef")
            nc.vector.tensor_copy(ef[:], ei[:])
            nc.vector.tensor_scalar(out=ef[:], in0=ef[:], scalar1=127.0,
                                    scalar2=None, op0=SUB)
            mb = fin.tile([128, 128], i32, name="mb", tag="mb")
            nc.vector.tensor_scalar(out=mb[:], in0=u, scalar1=0x007FFFFF,
                                    scalar2=0x3F800000, op0=AND, op1=OR)
            lnm = fin.tile([128, 128], f32, name="lnm", tag="lnm")
            nc.scalar.activation(lnm[:], mb[:].bitcast(f32), LN)
            lse = fin.tile([128, 128], f32, name="lse", tag="lse")
            nc.vector.scalar_tensor_tensor(
                out=lse[:], in0=ef[:], scalar=LN2, in1=lnm[:],
                op0=MUL, op1=ADD)
            acc = [fin.tile([128, 16], f32, name=f"acc{i}", tag=f"acc{i}")
                   for i in range(2)]
            nc.vector.memset(acc[0][:], 0.0)
            cur = 0
            for lh in range(8):
                nxt = 1 - cur
                nc.vector.scalar_tensor_tensor(
                    out=acc[nxt][:], in0=lse[:, lh * 16:lh * 16 + 16],
                    scalar=wb[:, lh:lh + 1], in1=acc[cur][:],
                    op0=MUL, op1=ADD)
                cur = nxt
            nc.sync.dma_start(out=out_d[:], in_=acc[cur][:])

    nc.compile()
    return nc


def _get_nc():
    if "nc" not in _CACHE:
        _CACHE["nc"] = _build()
    return _CACHE["nc"]


def kernel(g, W_lift, W_proj):
    from concourse.bass_utils import run_bass_kernel_spmd

    bf = ml_dtypes.bfloat16
    g = np.asarray(g, dtype=np.float32)
    W_lift = np.asarray(W_lift, dtype=np.float32)
    W_proj = np.asarray(W_proj, dtype=np.float32)

    nc = _get_nc()
    ident = np.eye(128, dtype=np.float32).astype(bf)
    uneg = (np.triu(np.full((128, 128), BIGNEG, dtype=np.float32), 1)
            ).astype(bf)
    w_all = W_proj.sum(axis=0).astype(np.float32)  # (H,)

    in_maps = []
    for core in range(NCORES):
        b, hg = core // 2, core % 2
        gt = g[b].T.reshape(NE, 128, 4, 512).transpose(1, 2, 0, 3)
        in_map = {
            "ident": ident,
            "uneg": uneg,
        }
        for tq in range(4):
            in_map[f"g_q{tq}"] = np.ascontiguousarray(gt[:, tq]).reshape(
                128, NE * 512).astype(bf)
        for p in range(PAIRS):
            h0 = hg * 8 + 2 * p
            h1 = h0 + 1
            rows = (list(range(h0 * D, h0 * D + D))
                    + list(range(h1 * D, h1 * D + D))
                    + list(range(E + h0 * D, E + h0 * D + D))
                    + list(range(E + h1 * D, E + h1 * D + D)))
            w_slice = W_lift[rows, :]  # (256 f, 1024 e)
            wp = np.ascontiguousarray(
                w_slice.T.reshape(NE, 128, 256).transpose(1, 0, 2)
            ).reshape(128, NE * 256).astype(bf)
            in_map[f"w_p{p}"] = wp
        wbv = np.broadcast_to(w_all[hg * 8: hg * 8 + 8],
                              (128, 8)).astype(np.float32)
        in_map["wb"] = np.ascontiguousarray(wbv)
        in_maps.append(in_map)

    res = run_bass_kernel_spmd(nc, in_maps, core_ids=list(range(NCORES)))
    _CACHE["last_results"] = res
    _CACHE["last_in_maps"] = in_maps

    out = np.zeros((B, T), dtype=np.float32)
    for core in range(NCORES):
        b = core // 2
        part = res.results[core]["out_part"]  # (128, 16)
        out[b] += part.T.reshape(-1)
    return out
